# revision 67
# baseline (speedup 1.0000x reference)
"""Trainium2 Bass kernel for nn_AlignModule_full (8 NeuronCores, data-parallel).

Sharding: core c = (q, h), q = c//2 flow batch, h = c%2 row half.
Each core computes flow(q, rows 64h..64h+64) from batch-q features, then
warps the 19 (n, ch) images with (3n+ch)%4 == q for its row half.

v2 pipeline: P3 (3x3 conv 64->2) is computed as 17 "pair" streams, each a
single 4-wide matmul producing flow rows (2k, 2k+1) from x-half-A and rows
(2k+30, 2k+31) from x-half-B simultaneously (one rhs stream, block lhsT).
Pairs interleave into the P2 loop so flow quarters finish early and the
per-quarter warp pipeline (flow->CL->idx->gather->weights->w4->combine)
overlaps with the remaining conv work.  Output is dumped raw and
reassembled on the host.
"""
import sys

for _p in ('/opt/trn_rl_repo',):
    if _p not in sys.path:
        sys.path.append(_p)

import numpy as np
import ml_dtypes

import concourse.bass as bass
import concourse.bacc as bacc
import concourse.mybir as mybir
import concourse.tile as tile

F32 = mybir.dt.float32
BF16 = mybir.dt.bfloat16
I16 = mybir.dt.int16
AF = mybir.ActivationFunctionType
ALU = mybir.AluOpType

H, W, CIN, T, CCLS, NB = 128, 256, 256, 64, 19, 4
SLAB_R = 68          # feature slab rows
WS = 258             # padded row width for t/x buffers
PAD0 = 2             # leading pad elems so every row start is 4B-aligned
XR = 66              # x rows total
XH = 36              # x rows per partition-half (A: 0..36, B: 30..66)
YS, XS = 76, 26      # gather slab rows/cols per (group, call=col-half)
LNUM = YS * XS       # base positions per partition
DCH = 8              # interleave chunk: 2 slots x (2x2 patch)
ROWB = 6             # slab row margin before first output row of the call
COLB = 5             # slab col margin before group col block
TCS = PAD0 + SLAB_R * WS   # t_cat partition stride
XSS = PAD0 + XH * WS       # x_sb partition stride

BF = ml_dtypes.bfloat16


def img_list(q):
    return [(n, ch) for n in range(NB) for ch in range(CCLS)
            if (3 * n + ch) % 4 == q]


def build_nc():
    nc = bacc.Bacc(None, target_bir_lowering=False, debug=False)
    P = nc.declare_dram_parameter
    f1_d = P("f1", [2, 128, SLAB_R, W], BF16, isOutput=False)
    f2_d = P("f2", [2, 128, SLAB_R, W], BF16, isOutput=False)
    wd_d = P("wd", [128, 2, 2, T], BF16, isOutput=False)
    wf1_d = P("wf1", [128, 9, T], BF16, isOutput=False)
    wf2p_d = P("wf2p", [128, 9, 4], BF16, isOutput=False)
    wf2s_d = P("wf2s", [128, 9, 4], BF16, isOutput=False)
    bn_d = P("bn", [128, 2, 1], F32, isOutput=False)
    mask_d = P("mask", [128, 2, 1], F32, isOutput=False)
    bx_d = P("bx", [128, 128], F32, isOutput=False)
    by_d = P("by", [128, 128], F32, isOutput=False)
    emat_d = P("emat", [8, 128], BF16, isOutput=False)
    dsrc_d = P("dsrc", [2, 128, LNUM * DCH], BF16, isOutput=False)
    out_d = P("out", [128, 4, 2, 512], F32, isOutput=True)

    # flow row-quarters, DRAM round-trip for CL transpose.  4 "homes":
    # partitions 0,1 = flow chs from A-side copies, 2,3 = from B-side.
    flow_dramq = [nc.dram_tensor(f"flow_q{q}", [4, W, 16], BF16)
                  for q in range(4)]

    NRB = 4               # feature rows per DMA batch
    NBATCH = (SLAB_R + NRB - 1) // NRB  # 17

    with tile.TileContext(nc) as tc:
        with (
            tc.tile_pool(name="stream", bufs=3) as sp,
            tc.tile_pool(name="big", bufs=1) as bp,
            tc.tile_pool(name="psA", bufs=2, space="PSUM") as pp,
        ):
            # ---- feature batches: [128, feat, ck, NRB, W] bf16 ring
            fts = {}

            def load_batch(b):
                r0 = NRB * b
                nr = min(NRB, SLAB_R - r0)
                ft = bp.tile([128, 2, 2, NRB, W], BF16, tag="fbatch",
                             name=f"fb{b}", bufs=3)
                for fi, fd in ((0, f1_d), (1, f2_d)):
                    src = bass.AP(tensor=fd, offset=r0 * W,
                                  ap=[[SLAB_R * W, 128], [128 * SLAB_R * W, 2],
                                      [W, nr], [1, W]])
                    (nc.sync if fi == 0 else nc.scalar).dma_start(
                        ft[:, fi, :, 0:nr, :], src)
                fts[b] = ft

            load_batch(0)

            # ---- constants (spread across both HWDGE queues) ----
            wd_s = bp.tile([128, 2, 2, T], BF16, tag="wd")
            wf1_s = bp.tile([128, 9, T], BF16, tag="wf1")
            wf2p_s = bp.tile([128, 9, 4], BF16, tag="wf2p")
            wf2s_s = bp.tile([128, 9, 4], BF16, tag="wf2s")
            bn_s = bp.tile([128, 2, 1], F32, tag="bn")
            mask_s = bp.tile([128, 2, 1], F32, tag="mask")
            emat_s = bp.tile([8, 128], BF16, tag="emat")
            bx_s = bp.tile([128, 128], F32, tag="bx")
            by_s = bp.tile([128, 128], F32, tag="by")
            for i, (t_, d_) in enumerate((
                    (wd_s, wd_d), (wf1_s, wf1_d), (wf2p_s, wf2p_d),
                    (wf2s_s, wf2s_d), (bn_s, bn_d), (mask_s, mask_d),
                    (emat_s, emat_d), (bx_s, bx_d), (by_s, by_d))):
                (nc.sync if i % 2 else nc.scalar).dma_start(t_[:], d_[:])

            # ---- big shared tiles ----
            t_cat = bp.tile([128, TCS], BF16, tag="tcat")
            x_sb = bp.tile([128, XSS], BF16, tag="xsb")
            dsrc = bp.tile([128, LNUM * DCH], BF16, tag="dsrc")
            dsrc2 = bp.tile([128, LNUM * DCH], BF16, tag="dsrc2")

            # gather ucode preload (dummy)
            dum_src = sp.tile([128, 8], BF16, tag="dumg", bufs=1)
            dum_idx = sp.tile([128, 1], I16, tag="dumi", bufs=1)
            dum_out = sp.tile([128, 32], BF16, tag="dumo", bufs=1)
            nc.vector.memset(dum_src[:], 0.0)
            nc.vector.memset(dum_idx[:], 0)
            nc.gpsimd.ap_gather(dum_out[:], dum_src[:], dum_idx[:],
                                channels=128, num_elems=4, d=2, num_idxs=16)
            # gather sources via software DGE: keeps the HWDGE queues free
            # for the feature stream, and the files land early
            nc.gpsimd.dma_start(dsrc[:], dsrc_d[0, :, :])
            nc.gpsimd.dma_start(dsrc2[:], dsrc_d[1, :, :])

            load_batch(1)

            # zero the pads: leading 2 elems + cols 256..257 of each row
            nc.vector.memset(
                bass.AP(tensor=t_cat.tensor, offset=0,
                        ap=[[TCS, 128], [1, PAD0]]), 0.0)
            nc.vector.memset(
                bass.AP(tensor=t_cat.tensor, offset=PAD0 + W,
                        ap=[[TCS, 128], [WS, SLAB_R], [1, 2]]), 0.0)
            nc.vector.memset(
                bass.AP(tensor=x_sb.tensor, offset=0,
                        ap=[[XSS, 128], [1, PAD0]]), 0.0)
            nc.vector.memset(
                bass.AP(tensor=x_sb.tensor, offset=PAD0 + W,
                        ap=[[XSS, 128], [WS, XH], [1, 2]]), 0.0)

            # ---- phase 1: 1x1 convs -> t_cat ----
            def p1_tile(it):
                r0 = 2 * it
                b, rr = r0 // NRB, r0 % NRB
                if rr == 0 and b + 1 < NBATCH and (b + 1) not in fts:
                    load_batch(b + 1)
                ft = fts[b]
                ps = pp.tile([128, 2 * W], F32, tag="pst", name="pst")
                for ck in range(2):
                    nc.tensor.matmul(ps[0:T, :], wd_s[:, 0, ck, :],
                                     ft[:, 0, ck, rr:rr + 2, :],
                                     start=(ck == 0), stop=(ck == 1),
                                     tile_position=(0, 0),
                                     skip_group_check=True)
                    nc.tensor.matmul(ps[T:128, :], wd_s[:, 1, ck, :],
                                     ft[:, 1, ck, rr:rr + 2, :],
                                     start=(ck == 0), stop=(ck == 1),
                                     tile_position=(0, 64),
                                     skip_group_check=True)
                dst = bass.AP(tensor=t_cat.tensor, offset=PAD0 + r0 * WS,
                              ap=[[TCS, 128], [WS, 2], [1, W]])
                src = ps[:].rearrange("p (r c) -> p r c", r=2, c=W)
                if it % 2:
                    nc.scalar.copy(dst, src)
                else:
                    nc.vector.tensor_copy(dst, src)

            # ---- phase 2: 3x3 conv 128->64 + BN + ReLU -> x_sb ----
            def p2_iter(it):
                jA = 2 * it
                jB = 30 + 2 * it
                ps = pp.tile([128, 2 * W], F32, tag="psx", name="psx")
                for tap in range(9):
                    dy, dx = tap // 3, tap % 3
                    rhsA = bass.AP(tensor=t_cat.tensor,
                                   offset=PAD0 + (jA + dy) * WS + dx - 1,
                                   ap=[[TCS, 128], [WS, 2], [1, W]])
                    rhsB = bass.AP(tensor=t_cat.tensor,
                                   offset=PAD0 + (jB + dy) * WS + dx - 1,
                                   ap=[[TCS, 128], [WS, 2], [1, W]])
                    nc.tensor.matmul(ps[0:T, :], wf1_s[:, tap, :], rhsA,
                                     start=(tap == 0), stop=(tap == 8),
                                     tile_position=(0, 0),
                                     skip_group_check=True)
                    nc.tensor.matmul(ps[T:128, :], wf1_s[:, tap, :], rhsB,
                                     start=(tap == 0), stop=(tap == 8),
                                     tile_position=(0, 64),
                                     skip_group_check=True)
                dst = bass.AP(tensor=x_sb.tensor, offset=PAD0 + (2 * it) * WS,
                              ap=[[XSS, 128], [WS, 2], [1, W]])
                nc.scalar.activation(dst,
                                     ps[:].rearrange("p (r c) -> p r c",
                                                     r=2, c=W),
                                     AF.Relu, bias=bn_s[:, 1],
                                     scale=bn_s[:, 0])

            # ---- phase 3: flow via pair streams ----
            btbig = {q: bp.tile([4, 16 * W], BF16, tag="btb",
                                name=f"btb{q}", bufs=3) for q in range(4)}

            def bt_copy(i0, src4, eng):
                qq, rr = i0 // 16, i0 % 16
                dst = bass.AP(tensor=btbig[qq].tensor, offset=rr,
                              ap=[[16 * W, 4], [1, 2], [16, W]])
                if eng is nc.vector:
                    nc.vector.tensor_copy(dst, src4)
                else:
                    nc.scalar.copy(dst, src4)

            def p3_pair(k):
                ps = pp.tile([128, 2 * W], F32,
                             tag=("psf" if k % 2 == 0 else "psfB"),
                             name="psf")
                pos = 32 * (k % 4)
                sl = ps[pos:pos + 4]
                for tap in range(9):
                    dy, dx = tap // 3, tap % 3
                    rhs = bass.AP(tensor=x_sb.tensor,
                                  offset=PAD0 + (2 * k + dy) * WS + dx - 1,
                                  ap=[[XSS, 128], [WS, 2], [1, W]])
                    nc.tensor.matmul(sl, wf2p_s[:, tap, :], rhs,
                                     start=(tap == 0), stop=(tap == 8),
                                     tile_position=(0, pos),
                                     skip_group_check=True)
                sv = sl.rearrange("p (r c) -> p r c", r=2, c=W)
                bt_copy(2 * k, sv, nc.vector)
                bt_copy(2 * k + 30, sv, nc.scalar)

            def p3_solo(i0, pos, eng, tag):
                ps = pp.tile([128, 2 * W], F32, tag=tag, name="psf")
                sl = ps[pos:pos + 4]
                for tap in range(9):
                    dy, dx = tap // 3, tap % 3
                    rhs = bass.AP(tensor=x_sb.tensor,
                                  offset=PAD0 + (i0 - 30 + dy) * WS + dx - 1,
                                  ap=[[XSS, 128], [WS, 2], [1, W]])
                    nc.tensor.matmul(sl, wf2s_s[:, tap, :], rhs,
                                     start=(tap == 0), stop=(tap == 8),
                                     tile_position=(0, pos),
                                     skip_group_check=True)
                bt_copy(i0, sl.rearrange("p (r c) -> p r c", r=2, c=W), eng)

            # ---- warp-side tiles ----
            cl_fx = bp.tile([128, 128], BF16, tag="clfx")
            cl_fy = bp.tile([128, 128], BF16, tag="clfy")

            def cl(tag):
                return bp.tile([128, 128], F32, tag=tag, name=tag)

            ix = cl("ix"); iy = cl("iy"); tmp = cl("tmp")
            x0i = bp.tile([128, 128], I16, tag="x0i")
            y0i = bp.tile([128, 128], I16, tag="y0i")
            x0f = cl("x0f"); y0f = cl("y0f")
            ef = cl("ef")
            eidx = bp.tile([128, 128], I16, tag="eidx")
            # bilinear weight planes, CL layout: [128, 4 corners, 128]
            wsall = bp.tile([128, 4, 128], BF16, tag="wsall")
            # broadcast-transposed weights: [8, (c 4)(w 2)(mm 16)(rl 16)]
            wgs = {}

            _qs = [nc.sync, nc.scalar]

            def cl_load(q):
                # partition home of flow ch: A-side copies land at 0,1;
                # B-side at 2,3.  q0 pure-A, q2/q3 pure-B, q1 mixed.
                for ch, dtile in ((0, cl_fx), (1, cl_fy)):
                    for w in range(2):
                        segs = ([(0, 16, ch)] if q == 0 else
                                [(0, 14, ch), (14, 2, 2 + ch)] if q == 1 else
                                [(0, 16, 2 + ch)])
                        for (r0, nr, home) in segs:
                            dst = bass.AP(tensor=dtile.tensor,
                                          offset=64 * w + 16 * q + r0,
                                          ap=[[128, 128], [1, nr]])
                            srcp = bass.AP(
                                tensor=flow_dramq[q],
                                offset=home * W * 16 + 16 * w * 16 + r0,
                                ap=[[32 * 16, 8], [16, 16], [1, nr]])
                            nc.sync.dma_start(dst, srcp)

            def idx_math(q):
                V = nc.vector

                def S(t):
                    return bass.AP(tensor=t.tensor, offset=16 * q,
                                   ap=[[128, 128], [64, 2], [1, 16]])

                V.tensor_scalar_mul(S(ix), S(cl_fx), 0.5)
                V.tensor_tensor(S(ix), S(ix), S(bx_s), ALU.add)
                V.tensor_scalar_mul(S(iy), S(cl_fy), 0.5)
                V.tensor_tensor(S(iy), S(iy), S(by_s), ALU.add)
                V.tensor_copy(S(x0i), S(ix))
                V.tensor_copy(S(x0f), S(x0i))
                V.tensor_tensor(S(tmp), S(x0f), S(ix), ALU.is_gt)
                V.tensor_tensor(S(x0f), S(x0f), S(tmp), ALU.subtract)
                V.tensor_copy(S(y0i), S(iy))
                V.tensor_copy(S(y0f), S(y0i))
                V.tensor_tensor(S(tmp), S(y0f), S(iy), ALU.is_gt)
                V.tensor_tensor(S(y0f), S(y0f), S(tmp), ALU.subtract)
                V.tensor_scalar_mul(S(ef), S(y0f), float(XS))
                V.tensor_tensor(S(ef), S(ef), S(x0f), ALU.add)
                V.tensor_copy(S(eidx), S(ef))

            def weights_math(q):
                # corners: 0:(x0,y0) 1:(x1,y0) 2:(x0,y1) 3:(x1,y1)
                V = nc.vector

                def S(t):
                    return bass.AP(tensor=t.tensor, offset=16 * q,
                                   ap=[[128, 128], [64, 2], [1, 16]])

                def SW(c):
                    return bass.AP(tensor=wsall.tensor, offset=c * 128 + 16 * q,
                                   ap=[[4 * 128, 128], [64, 2], [1, 16]])

                # ef is dead once eidx is cast; reuse it for 1-fy
                V.tensor_tensor(S(ix), S(ix), S(x0f), ALU.subtract)   # fx
                V.tensor_tensor(S(iy), S(iy), S(y0f), ALU.subtract)   # fy
                V.tensor_scalar(S(tmp), S(ix), -1.0, 1.0, ALU.mult, ALU.add)
                V.tensor_scalar(S(ef), S(iy), -1.0, 1.0, ALU.mult, ALU.add)
                V.tensor_tensor(SW(0), S(tmp), S(ef), ALU.mult)
                V.tensor_tensor(SW(1), S(ix), S(ef), ALU.mult)
                V.tensor_tensor(SW(2), S(tmp), S(iy), ALU.mult)
                V.tensor_tensor(SW(3), S(ix), S(iy), ALU.mult)

            def wg_dma(q):
                # [p=16G+mm, (c, 64w+16q+rl)] -> [G, (c, w, mm, rl)]
                w_g = bp.tile([8, 4 * 2 * 16 * 16], BF16, tag="wg",
                              name=f"wg{q}", bufs=2)
                for c in range(4):
                    for w in range(2):
                        dst = bass.AP(tensor=w_g.tensor,
                                      offset=c * 512 + w * 256,
                                      ap=[[4 * 512, 8], [16, 16], [1, 16]])
                        src = bass.AP(tensor=wsall.tensor,
                                      offset=16 * q + c * 128 + w * 64,
                                      ap=[[4 * 128, 128], [1, 16]])
                        nc.sync.dma_start(dst, src)
                wgs[q] = w_g

            gts = {}

            def warp_chain(q):
                nc.sync.dma_start(flow_dramq[q][:], btbig[q][:])
                cl_load(q)
                idx_math(q)
                weights_math(q)

            def warp_gathers(q):
                for w in range(2):
                    gt = bp.tile([128, 2048], BF16, tag="gat",
                                 name=f"g{q}{w}", bufs=6)
                    sl = slice(64 * w + 16 * q, 64 * w + 16 * q + 16)
                    nc.gpsimd.ap_gather(
                        gt[:], (dsrc if w == 0 else dsrc2)[:],
                        eidx[:, sl],
                        channels=128, num_elems=LNUM, d=DCH, num_idxs=256)
                    gts[(q, w)] = gt

            def warp_q(q):
                warp_chain(q)
                warp_gathers(q)

            # ---- emat broadcast + combine.  Vector path multiplies the
            # gather result by the weight chunks directly from PSUM;
            # the gpsimd path (q1) needs an SBUF w4 staging (scalar copies).
            def emat_rhs(q, call, c4):
                return bass.AP(tensor=wgs[q].tensor,
                               offset=call * 256 + c4 * 4,
                               ap=[[4 * 512, 8], [1, 4], [16, 16], [512, 4]])

            def tree_and_dump(q, call, g, eng):
                r1 = bp.tile([128, 1024], BF16, tag="r1",
                             name=f"r1{q}{call}", bufs=1)
                for half in range(2):
                    s0 = bass.AP(tensor=g.tensor, offset=2 * half,
                                 ap=[[2048, 128], [8, 256], [4, 2]])
                    s1 = bass.AP(tensor=g.tensor, offset=2 * half + 1,
                                 ap=[[2048, 128], [8, 256], [4, 2]])
                    dst = bass.AP(tensor=r1.tensor, offset=512 * half,
                                  ap=[[1024, 128], [2, 256], [1, 2]])
                    eng.tensor_tensor(dst, s0, s1, ALU.add)
                bb = bp.tile([128, 512], F32, tag="bb",
                             name=f"bb{q}{call}", bufs=1)
                eng.tensor_tensor(bb[:], r1[:, 0:512], r1[:, 512:1024],
                                  ALU.add)
                dstd = bass.AP(tensor=out_d, offset=q * 1024 + call * 512,
                               ap=[[4096, 128], [1, 512]])
                nc.sync.dma_start(dstd, bb[:])

            def emat_combine(q):
                wg_dma(q)
                for call in range(2):
                    g = gts[(q, call)]
                    for c4 in range(4):
                        pw = pp.tile([128, 512], F32, tag="pst", name="pw")
                        for s in range(2):
                            nc.tensor.matmul(pw[:, 256 * s:256 * (s + 1)],
                                             emat_s[:], emat_rhs(q, call, c4),
                                             start=True, stop=True,
                                             skip_group_check=True)
                        gch = bass.AP(tensor=g.tensor, offset=512 * c4,
                                      ap=[[2048, 128], [8, 64], [4, 2],
                                          [1, 4]])
                        pwv = bass.AP(tensor=pw.tensor, offset=0,
                                      ap=[[512, 128], [4, 64], [256, 2],
                                          [1, 4]])
                        nc.vector.tensor_tensor(gch, gch, pwv, ALU.mult)
                    tree_and_dump(q, call, g, nc.vector)



            # ================= emission schedule =================
            for it in range(18):
                p1_tile(it)
            # remaining feature batches + gather sources enqueued ahead of
            # any warp-chain DMA so a waiting DMA can't starve them
            for b in range(10, NBATCH):
                load_batch(b)
            for it in range(18):
                p2_iter(it)
                if it == 0:
                    # x row 0 (half A) is image row -1 for h=0: zero it
                    nc.vector.tensor_scalar_mul(
                        bass.AP(tensor=x_sb.tensor, offset=PAD0,
                                ap=[[XSS, T], [1, W]]),
                        bass.AP(tensor=x_sb.tensor, offset=PAD0,
                                ap=[[XSS, T], [1, W]]),
                        mask_s[0:T, 0])
                if it < 8:
                    p1_tile(18 + 2 * it)
                    p1_tile(19 + 2 * it)
                if it == 4:
                    for k in range(0, 4):
                        p3_pair(k)
                if it == 8:
                    for k in range(4, 8):
                        p3_pair(k)
                    warp_q(0)
                if it == 9:
                    p3_pair(8)
                    warp_q(2)
                if it == 15:
                    for k in range(9, 15):
                        p3_pair(k)
            # x half-B row 35 = x row 65 = image row 128 for h=1: zero it
            nc.vector.tensor_scalar_mul(
                bass.AP(tensor=x_sb.tensor, offset=T * XSS + PAD0 + 35 * WS,
                        ap=[[XSS, T], [1, W]]),
                bass.AP(tensor=x_sb.tensor, offset=T * XSS + PAD0 + 35 * WS,
                        ap=[[XSS, T], [1, W]]),
                mask_s[T:128, 1])
            p3_solo(60, 0, nc.vector, "psf")
            p3_solo(62, 32, nc.scalar, "psfB")
            warp_q(1)
            warp_chain(3)
            emat_combine(0)
            warp_gathers(3)
            emat_combine(2)
            emat_combine(1)
            emat_combine(3)
    nc.finalize()
    return nc


# ======================= host-side prep =======================

def _feat_slab(feat_b, h):
    """feat_b (256, 128, 256) f32 -> (2, 128, 68, 256) bf16 slab for half h."""
    r0 = 64 * h - 2
    slab = np.zeros((CIN, SLAB_R, W), np.float32)
    lo, hi = max(r0, 0), min(r0 + SLAB_R, H)
    slab[:, lo - r0:hi - r0, :] = feat_b[:, lo:hi, :]
    return np.ascontiguousarray(
        slab.reshape(2, 128, SLAB_R, W).astype(BF))


def _host_constants(q, h):
    R0 = 64 * h
    # CL layout: p = 16G + m, f = 64w + r; pixel (row R0+r, col 32G+16w+m)
    p = np.arange(128)[:, None]
    f = np.arange(128)[None, :]
    G = p // 16
    m = p % 16
    r = f % 64
    w = f // 64
    col = 32 * G + 16 * w + m
    row = R0 + r
    ix_base = col + col / (W - 1.0) - 0.5
    iy_base = row + row / (H - 1.0) - 0.5
    colbase = 32 * G + 16 * w - COLB
    rowbase = R0 - ROWB
    bx = np.broadcast_to(ix_base - colbase, (128, 128)).astype(np.float32).copy()
    by = np.broadcast_to(iy_base - rowbase, (128, 128)).astype(np.float32).copy()
    return bx, by


def _dsrc_build(pred_imgs, h):
    """pred_imgs: (19, 128, 256) f32. Returns (2, 128, LNUM*8) f32 gather
    source; call = col-half w, slab = rows [R0-6, R0+70) x 26-col band."""
    R0 = 64 * h
    padded = np.zeros((CCLS, H + 16, W + 16), np.float32)
    padded[:, 8:8 + H, 8:8 + W] = pred_imgs
    out = np.zeros((2, 128, LNUM, DCH), np.float32)
    rowbase = R0 - ROWB
    for call in range(2):
        for G in range(8):
            colbase = 32 * G + 16 * call - COLB
            for l in range(16):
                for slot in range(2):
                    img = l + 16 * slot
                    if img >= CCLS:
                        img = l
                    for j2 in range(2):
                        for j1 in range(2):
                            win = padded[img,
                                         8 + rowbase + j2: 8 + rowbase + j2 + YS,
                                         8 + colbase + j1: 8 + colbase + j1 + XS]
                            out[call, 16 * G + l, :, 4 * slot + 2 * j2 + j1] = \
                                win.reshape(-1)
    return out.reshape(2, 128, LNUM * DCH)


def make_inputs(core, t1_feature, t2_feature, t2_pred, w_down1, w_down2,
                w_flow1, bn_gamma, bn_beta, bn_mean, bn_var, w_flow2):
    q, h = core // 2, core % 2
    f1 = _feat_slab(t1_feature[q], h)
    f2 = _feat_slab(t2_feature[q], h)
    wd = np.stack([
        np.stack([w_down1[:, 128 * k:128 * (k + 1), 0, 0].T for k in range(2)]),
        np.stack([w_down2[:, 128 * k:128 * (k + 1), 0, 0].T for k in range(2)]),
    ]).transpose(2, 0, 1, 3).astype(BF).copy()        # (128,2,2,64)
    wf1 = np.stack([w_flow1[:, :, t // 3, t % 3].T for t in range(9)],
                   axis=1).astype(BF).copy()          # (128,9,64)
    wf2h = np.stack([w_flow2[:, :, t // 3, t % 3].T for t in range(9)],
                    axis=1).astype(BF)                # (64,9,2)
    wf2p = np.zeros((128, 9, 4), BF)
    wf2p[:T, :, 0:2] = wf2h
    wf2p[T:, :, 2:4] = wf2h
    wf2s = np.zeros((128, 9, 4), BF)
    wf2s[T:, :, 2:4] = wf2h
    scale = bn_gamma / np.sqrt(bn_var + 1e-5)
    bias = bn_beta - bn_mean * scale
    bn1 = np.stack([scale, bias], axis=1).reshape(T, 2, 1).astype(np.float32)
    bn = np.concatenate([bn1, bn1], axis=0)           # (128,2,1)
    mask = np.ones((128, 2, 1), np.float32)
    if h == 0:
        mask[0:T, 0] = 0.0   # x row 0 (half A) = image row -1
    else:
        mask[T:128, 1] = 0.0  # x half-B row 35 = x row 65 = image row 128
    bx, by = _host_constants(q, h)
    imgs = img_list(q)
    pred_imgs = np.stack([t2_pred[n, ch] for (n, ch) in imgs])
    dsrc = _dsrc_build(pred_imgs, h)
    emat = np.zeros((8, 128), BF)
    for Gi in range(8):
        emat[Gi, 16 * Gi:16 * (Gi + 1)] = 1.0
    return {
        "f1": f1, "f2": f2, "wd": wd, "wf1": wf1, "wf2p": wf2p, "wf2s": wf2s,
        "bn": bn, "mask": mask, "bx": bx, "by": by,
        "emat": emat, "dsrc": dsrc.astype(BF),
    }


def decode_out(o):
    """o: (128, 4, 2, 512) f32 raw dump -> (19, 64, 256)."""
    out = np.zeros((CCLS, 64, W), np.float32)
    arr = o.reshape(8, 16, 4, 2, 16, 16, 2)  # G, l, q, call, rl, mm, slot
    dst = out.reshape(CCLS, 4, 16, 8, 2, 16)  # img, q, rl, G, call, mm
    for slot, nl in ((0, 16), (1, 3)):
        sub = arr[:, :nl, :, :, :, :, slot]   # G, l, q, call, rl, mm
        dst[16 * slot:16 * slot + nl] = sub.transpose(1, 2, 4, 0, 3, 5)
    return out


_NC_CACHE = {}


def kernel(**inputs):
    from concourse.bass_utils import run_bass_kernel_spmd
    if "nc" not in _NC_CACHE:
        _NC_CACHE["nc"] = build_nc()
    nc = _NC_CACHE["nc"]
    in_maps = [make_inputs(c, **inputs) for c in range(8)]
    res = run_bass_kernel_spmd(nc, in_maps, list(range(8)))
    out = np.zeros((NB, CCLS, H, W), np.float32)
    for c in range(8):
        q, h = c // 2, c % 2
        o = decode_out(res.results[c]["out"])
        for i, (n, ch) in enumerate(img_list(q)):
            out[n, ch, 64 * h:64 * (h + 1), :] = o[i]
    return out
